# revision 8
# baseline (speedup 1.0000x reference)
"""Trainium2 Bass kernel for NeuronInvariantDeepSetLayer (segment_reduce).

kernel(**inputs) takes FULL unsharded inputs (as in reference.setup_inputs())
and returns the full [4096, 1] float32 output.

Strategy: data-parallel over 8 NeuronCores. Segments are split 512/core
(idx is sorted, so each core's rows are a contiguous slice of x). Rows are
host-padded so that each 128-segment block starts exactly at a 128-row tile
boundary -> every core runs the IDENTICAL instruction stream (pure SPMD),
only the data differs.

v2: fp8e4 DoubleRow everywhere the PE is hot. HW-measured (mb.py): a
DoubleRow LDW+MM pair with a FRESH stationary every matmul issues at the
same 82ns rate as bf16 while contracting K=256 -> mm1 is 3 MMs/tile instead
of 6, and the one-hot segment-reduce matmul covers TWO tiles per MM.
Per-tile PE cost drops ~580ns -> ~290ns. x is uploaded as fp8 (halves HBM
traffic vs bf16, which would otherwise become the bottleneck).

Numerics: w1 is host-scaled by 64 into e4m3's normal range (sigma 0.036*64
= 2.3); the relu psum->fp8 cast applies scale 1/64 so everything downstream
is in true units. sel one-hot entries are exact in fp8.

Algebraic restructure (as v1): segment-sum commutes with the linear second
phi layer, so W2@rho_w1 [192, 6] folds past the reduction:
    xsum1 = segsum(relu(x @ W1))
    out   = relu(xsum1 @ (W2 @ rho_w1) + count*(b2 @ rho_w1) + rb1) @ rho_w2 + rb2

Engine balance per pair of tiles (budget ~580ns): PE 7 MMs (bottleneck),
ACT relu tile0 (~290ns), DVE relu tile1 (~290ns), GpSimd sel pair (~300ns),
Sync queue: HWDGE x-chunk DMAs (1.5MB each).
"""

import sys
import os

sys.path.insert(0, "/opt/trn_rl_repo")

import numpy as np
import ml_dtypes

N = 400000
B = 4096
DIN = 768
DHID = 192
NCORES = 8
SPC = B // NCORES  # segments per core = 512
SBLK = 128  # segments per seg-block (psum accumulator width)
NBLK = SPC // SBLK  # 4 seg-blocks per core
P = 128
KP = DIN // 256  # 3 DoubleRow k-pairs for mm1
CH = 2048  # rows per full DMA chunk (16 tiles); last chunk may be 1024
TPC = CH // P  # tiles per full chunk = 16
TILEB = KP * 2 * P  # bytes per row per tile-layout = 768 (fp8)
SCALE = 64.0  # host-side w1 scale (undone at the relu cast)

f32 = np.float32
bf16 = ml_dtypes.bfloat16
f8 = ml_dtypes.float8_e4m3

# walrus --enable-ldw-opt=true (bass default false). Safe here: this kernel
# emits no transpose-mode ldweights (which that pass can't handle).
USE_LDW_OPT = True
DEFER_PAIRS = 4  # seg MM deferral (pairs) so relu results are ready


def _prep(x, idx):
    """Host-side sharding + DoubleRow tile layout + fp8 cast.

    Returns xs_t[c]: [P, ntiles*TILEB] fp8 where
    xs_t[c][p][((n*KP + c3)*2 + i)*128 + r] = x_row(c, n*128 + r)[(2*c3+i)*128 + p],
    ixs_arr[c]: [P, ntiles] u8 local-in-block segment ids (255 padding),
    plus tblk and segment counts.
    """
    if np.any(np.diff(idx) < 0):  # defensive: spec says idx is sorted
        order = np.argsort(idx, kind="stable")
        x, idx = x[order], idx[order]
    counts = np.bincount(idx, minlength=B)
    assert counts.sum() == x.shape[0]
    bounds = np.concatenate([[0], np.cumsum(counts)]).astype(np.int64)
    # Segment-to-block assignment is free (local ids + an output gather), so
    # LPT-balance segments across the NCORES*NBLK bins of SBLK segments each.
    nbins = NCORES * NBLK
    sorder = np.argsort(-counts, kind="stable")
    bin_rows = np.zeros(nbins, np.int64)
    bin_nseg = np.zeros(nbins, np.int32)
    bin_segs = [[] for _ in range(nbins)]
    for sg in sorder:
        cand = np.flatnonzero(bin_nseg < SBLK)
        b = int(cand[np.argmin(bin_rows[cand])])
        bin_segs[b].append(int(sg))
        bin_rows[b] += counts[sg]
        bin_nseg[b] += 1
    tblk = int(np.ceil(bin_rows.max() / P))
    tblk = ((tblk + 1) // 2) * 2  # even -> pairs of tiles stay in one block
    NP = NBLK * tblk * P
    ntiles = NP // P
    x8 = x.astype(f8)
    xs_t = np.zeros((NCORES, P, ntiles * TILEB), f8)
    ixs = np.full((NCORES, NP), 255, np.uint8)
    # loc[seg] = flat output position (core*SPC + blk*SBLK + slot)
    loc = np.zeros(B, np.int64)
    for c in range(NCORES):
        xc = np.zeros((NP, DIN), f8)
        for blk in range(NBLK):
            b = c * NBLK + blk
            d = blk * tblk * P
            for j, sg in enumerate(bin_segs[b]):
                loc[sg] = c * SPC + blk * SBLK + j
                r0, r1 = int(bounds[sg]), int(bounds[sg + 1])
                if r1 > r0:
                    xc[d : d + (r1 - r0)] = x8[r0:r1]
                    ixs[c, d : d + (r1 - r0)] = j
                    d += r1 - r0
        # [n, r, c3, i, p] -> [p, n, c3, i, r]
        xs_t[c] = (
            xc.reshape(ntiles, P, KP, 2, P)
            .transpose(4, 0, 2, 3, 1)
            .reshape(P, ntiles * TILEB)
        )
    # ixs_arr[c, p, n] = ixs[c, n*P + p]
    ixs_arr = np.ascontiguousarray(
        ixs.reshape(NCORES, ntiles, P).transpose(0, 2, 1)
    )
    return xs_t, ixs_arr, tblk, counts, loc


def _build(tblk, phi_w1, phi_b1, phi_w2, phi_b2, rho_w1, rho_b1, rho_w2, rho_b2):
    import concourse.bacc as bacc
    import concourse.mybir as mybir
    import concourse.tile as tile

    BF = mybir.dt.bfloat16
    F8 = mybir.dt.float8e4
    F32 = mybir.dt.float32
    U8 = mybir.dt.uint8
    DR = mybir.MatmulPerfMode.DoubleRow
    Relu = mybir.ActivationFunctionType.Relu
    Copy = mybir.ActivationFunctionType.Copy
    Alu = mybir.AluOpType

    has_b1 = bool(np.any(phi_b1 != 0))
    has_b2 = bool(np.any(phi_b2 != 0))
    has_rb1 = bool(np.any(rho_b1 != 0))
    has_rb2 = bool(np.any(rho_b2 != 0))
    HW = DHID + 1 if has_b2 else DHID  # h1b/pseg width (count col when b2!=0)

    # ---- packed constants (inlined into the NEFF) ----
    # w1dr[p, c3, i, h] = W1[(2*c3+i)*128+p, h] * SCALE, fp8
    w1s_np = np.clip(phi_w1.astype(np.float64) * SCALE, -240, 240)
    w1dr = np.ascontiguousarray(
        w1s_np.reshape(KP, 2, P, DHID).transpose(2, 0, 1, 3)
    ).astype(f8)
    # no nonlinearity sits between W2 and rho_w1, so fold them on the host:
    wfold = (phi_w2.astype(np.float64) @ rho_w1.astype(np.float64)).astype(f32)
    wf0 = np.ascontiguousarray(wfold[0:P, :]).astype(bf16)  # [128, 6]
    wf1 = np.ascontiguousarray(wfold[P:DHID, :]).astype(bf16)  # [64, 6]
    b2f = np.ascontiguousarray((phi_b2 @ rho_w1).reshape(1, 6)).astype(bf16)
    rw2k = np.ascontiguousarray(rho_w2).astype(bf16)  # [6, 1]
    idn16 = np.eye(P, dtype=bf16)
    # js4[p, i, r] = r  (iota for the quad is_equal)
    js2 = np.ascontiguousarray(
        np.broadcast_to(np.arange(SBLK, dtype=np.uint8)[None, None, :], (P, 4, SBLK))
    )
    b1row = np.ascontiguousarray(phi_b1.reshape(1, DHID) * SCALE).astype(f32)
    ones1 = np.ones((1, P), f32)
    onesc = np.ones((P, 1), f8)
    rb1k = np.ascontiguousarray(rho_b1.reshape(6, 1)).astype(f32)
    rb2k = np.ascontiguousarray(rho_b2.reshape(1, 1)).astype(f32)

    NP = NBLK * tblk * P
    ntiles = NP // P
    npairs = ntiles // 2
    # chunk row-splits: full 2048-row chunks, a trailing 1024 if needed
    nfull, rem = divmod(NP, CH)
    chunk_tiles = [TPC] * nfull + ([rem // P] if rem else [])

    nc = bacc.Bacc(None, target_bir_lowering=False)
    x_in = nc.dram_tensor("xt_shard", [P, ntiles * TILEB], F8, kind="ExternalInput")
    ix_in = nc.dram_tensor("idxlf", [P, ntiles], U8, kind="ExternalInput")
    out_d = nc.dram_tensor("out_shard", [SPC], F32, kind="ExternalOutput")

    w1d = nc.inline_tensor(w1dr, "w1dr")
    wf0d = nc.inline_tensor(wf0, "wf0")
    wf1d = nc.inline_tensor(wf1, "wf1")
    b2fd = nc.inline_tensor(b2f, "b2f") if has_b2 else None
    rw2d = nc.inline_tensor(rw2k, "rw2k")
    idn16d = nc.inline_tensor(idn16, "idn16")
    js2d = nc.inline_tensor(js2, "js2")
    b1d = nc.inline_tensor(b1row, "b1row") if has_b1 else None
    ones1d = nc.inline_tensor(ones1, "ones1") if has_b1 else None
    onescd = nc.inline_tensor(onesc, "onesc") if has_b2 else None
    rb1d = nc.inline_tensor(rb1k, "rb1k") if has_rb1 else None
    rb2d = nc.inline_tensor(rb2k, "rb2k") if has_rb2 else None

    with tile.TileContext(nc) as tc:
        with (
            tc.tile_pool(name="consts", bufs=1) as cpool,
            tc.tile_pool(name="xb", bufs=8) as xpool,
            tc.tile_pool(name="ixb", bufs=1) as ixpool,
            tc.tile_pool(name="h1b", bufs=6) as h1pool,
            tc.tile_pool(name="selb", bufs=6) as selpool,
            tc.tile_pool(name="rho", bufs=2) as rhopool,
            tc.tile_pool(name="ph1", bufs=3, space="PSUM") as ph1,
            tc.tile_pool(name="pseg", bufs=1, space="PSUM") as pseg,
            tc.tile_pool(name="prho", bufs=1, space="PSUM") as prho,
        ):
            # ---- early-needed data. idn16 (warmup dep) first on the sync
            # (HWDGE) ring, then w1dr, then the x chunks. Small/late consts
            # ride the gpsimd (SWDGE) ring.
            idn16s = cpool.tile([P, P], BF, tag="idn", name="idn16s")
            nc.sync.dma_start(out=idn16s[:], in_=idn16d[:])
            w1s = cpool.tile([P, KP, 2, DHID], F8, tag="w1s", name="w1s")
            nc.sync.dma_start(out=w1s[:], in_=w1d[:])
            ixall = ixpool.tile([P, ntiles], U8, tag="ixall", name="ixall")
            nc.gpsimd.dma_start(out=ixall[:], in_=ix_in[:])
            js = cpool.tile([P, 4, SBLK], U8, tag="js", name="js")
            nc.gpsimd.dma_start(out=js[:], in_=js2d[:])
            wf0s = cpool.tile_from(wf0d[:], forced_dma_engine=mybir.EngineType.Pool)
            wf1s = cpool.tile_from(wf1d[:], forced_dma_engine=mybir.EngineType.Pool)
            b2fs = cpool.tile_from(b2fd[:], forced_dma_engine=mybir.EngineType.Pool) if has_b2 else None
            rw2s = cpool.tile_from(rw2d[:], forced_dma_engine=mybir.EngineType.Pool)
            b1s = cpool.tile_from(b1d[:], forced_dma_engine=mybir.EngineType.Pool) if has_b1 else None
            ones1s = cpool.tile_from(ones1d[:], forced_dma_engine=mybir.EngineType.Pool) if has_b1 else None
            onescs = cpool.tile_from(onescd[:], forced_dma_engine=mybir.EngineType.Pool) if has_b2 else None
            rb1s = cpool.tile_from(rb1d[:], forced_dma_engine=mybir.EngineType.Pool) if has_rb1 else None
            rb2s = cpool.tile_from(rb2d[:], forced_dma_engine=mybir.EngineType.Pool) if has_rb2 else None

            # warm up the PE HAM clock gate while the first x chunks stream
            # in: wide dummy matmuls (N=512) so ~3.4us of PE busy time costs
            # few instructions and starts as soon as idn16 lands.
            # single psum bank shared by warmup + all rho scratch:
            # [:, 0:2, :] = xsum transpose, [0:6, 2, :] = rho hidden,
            # [0:1, 3, :] = rho output
            rhot = prho.tile([P, 4, P], F32, tag="rho", name="rhot")
            for _ in range(9):
                nc.tensor.matmul(
                    out=rhot[:],
                    lhsT=idn16s[:],
                    rhs=idn16s[:].to_broadcast([P, P, 4]),
                    start=True,
                    stop=True,
                )
            # one psum bank, two block-alternating segment accumulators
            psegT = pseg.tile([P, 2, HW], F32, tag="seg", name="psegT")

            pending = []  # deferred seg matmuls: (selb, h1b, pr)
            rho_q = []  # deferred rho stages: callables

            def emit_seg(selq_t, h1b_t, g, pr):
                blk = pr // (tblk // 2)
                nc.tensor.matmul(
                    out=psegT[:, blk % 2, :],
                    lhsT=selq_t[:, 2 * g : 2 * g + 2, :],
                    rhs=h1b_t[:, 2 * g : 2 * g + 2, :],
                    start=(pr % (tblk // 2) == 0),
                    stop=(pr % (tblk // 2) == (tblk // 2) - 1),
                    perf_mode=DR,
                )
                if pr % (tblk // 2) == (tblk // 2) - 1:
                    stage_rho(blk, psegT[:, blk % 2, :])

            def stage_rho(blk, pseg_t):
                # stage A: psum -> bf16, transpose via identity matmuls
                xsb = rhopool.tile([P, HW], BF, tag="xsb")
                xsT = rhopool.tile([P, 2, P], BF, tag="xsT")

                def stage_a0():
                    nc.vector.tensor_copy(out=xsb[:], in_=pseg_t[:])

                def stage_a1():
                    nc.tensor.matmul(
                        out=rhot[:, 0, :], lhsT=xsb[:, 0:P], rhs=idn16s[:],
                        start=True, stop=True,
                    )
                    nc.tensor.matmul(
                        out=rhot[0 : HW - P, 1, :], lhsT=xsb[:, P:HW], rhs=idn16s[:],
                        start=True, stop=True,
                    )

                def stage_b():
                    nc.vector.tensor_copy(out=xsT[:, 0, :], in_=rhot[:, 0, :])
                    nc.vector.tensor_copy(
                        out=xsT[0 : HW - P, 1, :], in_=rhot[0 : HW - P, 1, :]
                    )

                def stage_c():
                    # r [6, 128 segs] = relu((W2@rho_w1).T @ xsum1T (+ b2f*cnt) + rb1)
                    prT = rhot[0:6, 2, :]
                    nc.tensor.matmul(
                        out=prT, lhsT=wf0s[:], rhs=xsT[:, 0, :],
                        start=True, stop=False,
                    )
                    nc.tensor.matmul(
                        out=prT, lhsT=wf1s[:], rhs=xsT[0:64, 1, :],
                        start=False, stop=not has_b2,
                    )
                    if has_b2:
                        nc.tensor.matmul(
                            out=prT, lhsT=b2fs[:], rhs=xsT[64:65, 1, :],
                            start=False, stop=True,
                        )
                    rtb = rhopool.tile([6, P], BF, tag="rtb")
                    if has_rb1:
                        nc.scalar.activation(
                            out=rtb[:], in_=prT, func=Relu, bias=rb1s[:]
                        )
                    else:
                        nc.scalar.activation(out=rtb[:], in_=prT, func=Relu)
                    pot = rhot[0:1, 3, :]
                    nc.tensor.matmul(
                        out=pot, lhsT=rw2s[:], rhs=rtb[:], start=True, stop=True
                    )
                    ob = rhopool.tile([1, P], F32, tag="ob")
                    if has_rb2:
                        nc.scalar.activation(out=ob[:], in_=pot, func=Copy, bias=rb2s[:])
                    else:
                        nc.scalar.activation(out=ob[:], in_=pot, func=Copy)
                    nc.sync.dma_start(
                        out=out_d[blk * SBLK : (blk + 1) * SBLK], in_=ob[:]
                    )

                rho_q.append(stage_a0)
                rho_q.append(stage_a1)
                rho_q.append(stage_b)
                rho_q.append(stage_c)

            toff = 0  # tile offset of current chunk
            boff = 0  # byte offset into x_in per partition
            for ch, ctiles in enumerate(chunk_tiles):
                # ramp: first chunk in pieces so the PE starts early while
                # keeping per-partition DMA lines >= 768B
                xb = xpool.tile([P, TPC, KP, 2, P], F8, tag="xb", name=f"xb_{ch}")
                if ch == 0:
                    splits = [1, 1, 2, 4] + ([8] if ctiles == TPC else [])
                elif ch < 4:
                    splits = [8, 8] if ctiles == TPC else [8]
                else:
                    splits = [ctiles]
                n0 = 0
                for tq in splits:
                    nc.sync.dma_start(
                        out=xb[:, n0 : n0 + tq, :, :, :],
                        in_=x_in[
                            :, boff + n0 * TILEB : boff + (n0 + tq) * TILEB
                        ].rearrange("p (n c3 i r) -> p n c3 i r", n=tq, c3=KP, i=2),
                    )
                    n0 += tq

                for q in range(ctiles // 4):
                    t0 = toff + q * 4  # global tile index of quad start
                    pr0 = t0 // 2  # global pair index
                    ph1t = ph1.tile([P, 4, 256], F32, tag="h1", name=f"ph1_{t0}")
                    for j in range(4):
                        n = q * 4 + j
                        for c3 in range(KP):
                            nc.tensor.matmul(
                                out=ph1t[:, j, 0:DHID],
                                lhsT=xb[:, n, c3, :, :],
                                rhs=w1s[:, c3, :, :],
                                start=(c3 == 0),
                                stop=(c3 == KP - 1 and not has_b1),
                                perf_mode=DR,
                            )
                        if has_b1:
                            nc.tensor.matmul(
                                out=ph1t[:, j, 0:DHID], lhsT=ones1s[:], rhs=b1s[:],
                                start=False, stop=True,
                            )
                        # interleave deferred seg MMs / rho stages between
                        # mm1 groups so their input latencies are hidden
                        if j % 2 == 0:
                            if len(pending) > DEFER_PAIRS:
                                emit_seg(*pending.pop(0))
                        elif rho_q:
                            rho_q.pop(0)()
                    h1b = h1pool.tile([P, 4, HW], F8, tag="h1b", name=f"h1b_{t0}")
                    # relu + 1/SCALE + fp8 cast, one ACT instr per quad
                    nc.scalar.activation(
                        out=h1b[:, :, 0:DHID], in_=ph1t[:, :, 0:DHID], func=Relu,
                        scale=1.0 / SCALE,
                    )
                    if has_b2:
                        nc.vector.tensor_copy(
                            out=h1b[:, :, DHID:HW],
                            in_=onescs[:].to_broadcast([P, 4, 1]),
                        )
                    selq = selpool.tile([P, 4, SBLK], F8, tag="selb", name=f"sel_{t0}")
                    nc.vector.tensor_tensor(
                        out=selq[:],
                        in0=ixall[:, t0 : t0 + 4].to_broadcast([P, 4, SBLK]),
                        in1=js[:],
                        op=Alu.is_equal,
                    )
                    pending.append((selq, h1b, 0, pr0))
                    pending.append((selq, h1b, 1, pr0 + 1))
                toff += ctiles
                boff += ctiles * TILEB
            while pending:
                emit_seg(*pending.pop(0))
            while rho_q:
                rho_q.pop(0)()

    nc.compile()
    return nc


_CACHE = {}


def _get_nc(tblk, weights):
    key = tblk
    if key not in _CACHE:
        if USE_LDW_OPT:
            import concourse.bass_utils as bu

            orig = bu.run_command

            def run_command_ldwopt(argv, **kw):
                argv = [
                    "--enable-ldw-opt=true" if a == "--enable-ldw-opt=false" else a
                    for a in argv
                ]
                return orig(argv, **kw)

            bu.run_command = run_command_ldwopt
            try:
                _CACHE[key] = _build(tblk, *weights)
            finally:
                bu.run_command = orig
        else:
            _CACHE[key] = _build(tblk, *weights)
    return _CACHE[key]


def _run(inputs, trace=False):
    from concourse.bass_utils import run_bass_kernel_spmd

    inp = {k: np.asarray(v) for k, v in inputs.items()}
    x = inp["x"].astype(f32, copy=False)
    idx = inp["idx"].astype(np.int32, copy=False)
    weights = tuple(
        inp[k].astype(f32, copy=False)
        for k in ("phi_w1", "phi_b1", "phi_w2", "phi_b2", "rho_w1", "rho_b1", "rho_w2", "rho_b2")
    )
    xs_t, ixs, tblk, counts, loc = _prep(x, idx)
    nc = _get_nc(tblk, weights)
    in_maps = [{"xt_shard": xs_t[c], "idxlf": ixs[c]} for c in range(NCORES)]
    res = run_bass_kernel_spmd(nc, in_maps, core_ids=list(range(NCORES)), trace=trace)
    res_flat = np.concatenate([res.results[c]["out_shard"] for c in range(NCORES)])
    out = res_flat[loc].reshape(B, 1).astype(f32)
    # safety net: empty segments (never happens for the target distribution)
    if np.any(counts == 0):
        (phi_w1, phi_b1, phi_w2, phi_b2, rho_w1, rho_b1, rho_w2, rho_b2) = weights
        z = np.zeros((1, DHID), f32)
        r = np.maximum(z @ rho_w1 + rho_b1, 0.0)
        o0 = (r @ rho_w2 + rho_b2).astype(f32)
        out[counts == 0] = o0
    return out, res


def kernel(**inputs) -> np.ndarray:
    return _run(inputs, trace=False)[0]


if __name__ == "__main__":
    # quick self-test against numpy
    rng = np.random.default_rng(0)
    x = rng.standard_normal((N, DIN)).astype(f32)
    idx = np.sort(rng.integers(0, B, N).astype(np.int32))
    w1 = (rng.standard_normal((DIN, DHID)) / np.sqrt(DIN)).astype(f32)
    w2 = (rng.standard_normal((DHID, DHID)) / np.sqrt(DHID)).astype(f32)
    r1 = (rng.standard_normal((DHID, 6)) / np.sqrt(DHID)).astype(f32)
    r2 = (rng.standard_normal((6, 1)) / np.sqrt(6)).astype(f32)
    inputs = dict(
        x=x, idx=idx,
        phi_w1=w1, phi_b1=np.zeros(DHID, f32), phi_w2=w2, phi_b2=np.zeros(DHID, f32),
        rho_w1=r1, rho_b1=np.zeros(6, f32), rho_w2=r2, rho_b2=np.zeros(1, f32),
    )
    out = kernel(**inputs)
    h = np.maximum(x @ w1, 0.0) @ w2
    xsum = np.zeros((B, DHID), f32)
    np.add.at(xsum, idx, h)
    exp = np.maximum(xsum @ r1, 0.0) @ r2
    rel = np.linalg.norm(out - exp) / np.linalg.norm(exp)
    print("self-test rel err:", rel)


# revision 9
# speedup vs baseline: 1.0390x; 1.0390x over previous
"""Trainium2 Bass kernel for NeuronInvariantDeepSetLayer (segment_reduce).

kernel(**inputs) takes FULL unsharded inputs (as in reference.setup_inputs())
and returns the full [4096, 1] float32 output.

Strategy: data-parallel over 8 NeuronCores. Segments are split 512/core
(idx is sorted, so each core's rows are a contiguous slice of x). Rows are
host-padded so that each 128-segment block starts exactly at a 128-row tile
boundary -> every core runs the IDENTICAL instruction stream (pure SPMD),
only the data differs.

v2: fp8e4 DoubleRow everywhere the PE is hot. HW-measured (mb.py): a
DoubleRow LDW+MM pair with a FRESH stationary every matmul issues at the
same 82ns rate as bf16 while contracting K=256 -> mm1 is 3 MMs/tile instead
of 6, and the one-hot segment-reduce matmul covers TWO tiles per MM.
Per-tile PE cost drops ~580ns -> ~290ns. x is uploaded as fp8 (halves HBM
traffic vs bf16, which would otherwise become the bottleneck).

Numerics: w1 is host-scaled by 64 into e4m3's normal range (sigma 0.036*64
= 2.3); the relu psum->fp8 cast applies scale 1/64 so everything downstream
is in true units. sel one-hot entries are exact in fp8.

Algebraic restructure (as v1): segment-sum commutes with the linear second
phi layer, so W2@rho_w1 [192, 6] folds past the reduction:
    xsum1 = segsum(relu(x @ W1))
    out   = relu(xsum1 @ (W2 @ rho_w1) + count*(b2 @ rho_w1) + rb1) @ rho_w2 + rb2

Engine balance per pair of tiles (budget ~580ns): PE 7 MMs (bottleneck),
ACT relu tile0 (~290ns), DVE relu tile1 (~290ns), GpSimd sel pair (~300ns),
Sync queue: HWDGE x-chunk DMAs (1.5MB each).
"""

import sys
import os

sys.path.insert(0, "/opt/trn_rl_repo")

import numpy as np
import ml_dtypes

N = 400000
B = 4096
DIN = 768
DHID = 192
NCORES = 8
SPC = B // NCORES  # segments per core = 512
SBLK = 128  # segments per seg-block (psum accumulator width)
NBLK = SPC // SBLK  # 4 seg-blocks per core
P = 128
KP = DIN // 256  # 3 DoubleRow k-pairs for mm1
CH = 2048  # rows per full DMA chunk (16 tiles); last chunk may be 1024
TPC = CH // P  # tiles per full chunk = 16
TILEB = KP * 2 * P  # bytes per row per tile-layout = 768 (fp8)
SCALE = 64.0  # host-side w1 scale (undone at the relu cast)

f32 = np.float32
bf16 = ml_dtypes.bfloat16
f8 = ml_dtypes.float8_e4m3

# walrus --enable-ldw-opt=true (bass default false). Safe here: this kernel
# emits no transpose-mode ldweights (which that pass can't handle).
USE_LDW_OPT = True
DEFER_PAIRS = 4  # seg MM deferral (pairs) so relu results are ready


def _prep(x, idx):
    """Host-side sharding + DoubleRow tile layout + fp8 cast.

    Returns xs_t[c]: [P, ntiles*TILEB] fp8 where
    xs_t[c][p][((n*KP + c3)*2 + i)*128 + r] = x_row(c, n*128 + r)[(2*c3+i)*128 + p],
    ixs_arr[c]: [P, ntiles] u8 local-in-block segment ids (255 padding),
    plus tblk and segment counts.
    """
    if np.any(np.diff(idx) < 0):  # defensive: spec says idx is sorted
        order = np.argsort(idx, kind="stable")
        x, idx = x[order], idx[order]
    counts = np.bincount(idx, minlength=B)
    assert counts.sum() == x.shape[0]
    bounds = np.concatenate([[0], np.cumsum(counts)]).astype(np.int64)
    # Segment-to-block assignment is free (local ids + an output gather), so
    # LPT-balance segments across the NCORES*NBLK bins of SBLK segments each.
    nbins = NCORES * NBLK
    sorder = np.argsort(-counts, kind="stable")
    bin_rows = np.zeros(nbins, np.int64)
    bin_nseg = np.zeros(nbins, np.int32)
    bin_segs = [[] for _ in range(nbins)]
    for sg in sorder:
        cand = np.flatnonzero(bin_nseg < SBLK)
        b = int(cand[np.argmin(bin_rows[cand])])
        bin_segs[b].append(int(sg))
        bin_rows[b] += counts[sg]
        bin_nseg[b] += 1
    tblk = int(np.ceil(bin_rows.max() / P))
    tblk = ((tblk + 1) // 2) * 2  # even -> pairs of tiles stay in one block
    NP = NBLK * tblk * P
    ntiles = NP // P
    x8 = x.astype(f8)
    xs_t = np.zeros((NCORES, P, ntiles * TILEB), f8)
    ixs = np.full((NCORES, NP), 255, np.uint8)
    # loc[seg] = flat output position (core*SPC + blk*SBLK + slot)
    loc = np.zeros(B, np.int64)
    for c in range(NCORES):
        xc = np.zeros((NP, DIN), f8)
        for blk in range(NBLK):
            b = c * NBLK + blk
            d = blk * tblk * P
            for j, sg in enumerate(bin_segs[b]):
                loc[sg] = c * SPC + blk * SBLK + j
                r0, r1 = int(bounds[sg]), int(bounds[sg + 1])
                if r1 > r0:
                    xc[d : d + (r1 - r0)] = x8[r0:r1]
                    ixs[c, d : d + (r1 - r0)] = j
                    d += r1 - r0
        # [n, r, c3, i, p] -> [p, n, c3, i, r]
        xs_t[c] = (
            xc.reshape(ntiles, P, KP, 2, P)
            .transpose(4, 0, 2, 3, 1)
            .reshape(P, ntiles * TILEB)
        )
    # ixs_arr[c, p, n] = ixs[c, n*P + p]
    ixs_arr = np.ascontiguousarray(
        ixs.reshape(NCORES, ntiles, P).transpose(0, 2, 1)
    )
    return xs_t, ixs_arr, tblk, counts, loc


def _build(tblk, phi_w1, phi_b1, phi_w2, phi_b2, rho_w1, rho_b1, rho_w2, rho_b2):
    import concourse.bacc as bacc
    import concourse.mybir as mybir
    import concourse.tile as tile

    BF = mybir.dt.bfloat16
    F8 = mybir.dt.float8e4
    F32 = mybir.dt.float32
    U8 = mybir.dt.uint8
    DR = mybir.MatmulPerfMode.DoubleRow
    Relu = mybir.ActivationFunctionType.Relu
    Copy = mybir.ActivationFunctionType.Copy
    Alu = mybir.AluOpType

    has_b1 = bool(np.any(phi_b1 != 0))
    has_b2 = bool(np.any(phi_b2 != 0))
    has_rb1 = bool(np.any(rho_b1 != 0))
    has_rb2 = bool(np.any(rho_b2 != 0))
    HW = DHID + 1 if has_b2 else DHID  # h1b/pseg width (count col when b2!=0)

    # ---- packed constants (inlined into the NEFF) ----
    # w1dr[p, c3, i, h] = W1[(2*c3+i)*128+p, h] * SCALE, fp8
    w1s_np = np.clip(phi_w1.astype(np.float64) * SCALE, -240, 240)
    w1dr = np.ascontiguousarray(
        w1s_np.reshape(KP, 2, P, DHID).transpose(2, 0, 1, 3)
    ).astype(f8)
    # no nonlinearity sits between W2 and rho_w1, so fold them on the host:
    wfold = (phi_w2.astype(np.float64) @ rho_w1.astype(np.float64)).astype(f32)
    wf0 = np.ascontiguousarray(wfold[0:P, :]).astype(bf16)  # [128, 6]
    wf1 = np.ascontiguousarray(wfold[P:DHID, :]).astype(bf16)  # [64, 6]
    b2f = np.ascontiguousarray((phi_b2 @ rho_w1).reshape(1, 6)).astype(bf16)
    rw2k = np.ascontiguousarray(rho_w2).astype(bf16)  # [6, 1]
    idn16 = np.eye(P, dtype=bf16)
    # js4[p, i, r] = r  (iota for the quad is_equal)
    js2 = np.ascontiguousarray(
        np.broadcast_to(np.arange(SBLK, dtype=np.uint8)[None, None, :], (P, 4, SBLK))
    )
    b1row = np.ascontiguousarray(phi_b1.reshape(1, DHID) * SCALE).astype(f32)
    ones1 = np.ones((1, P), f32)
    onesc = np.ones((P, 1), f8)
    rb1k = np.ascontiguousarray(rho_b1.reshape(6, 1)).astype(f32)
    rb2k = np.ascontiguousarray(rho_b2.reshape(1, 1)).astype(f32)

    NP = NBLK * tblk * P
    ntiles = NP // P
    npairs = ntiles // 2
    # chunk row-splits: full 2048-row chunks, a trailing 1024 if needed
    nfull, rem = divmod(NP, CH)
    chunk_tiles = [TPC] * nfull + ([rem // P] if rem else [])

    nc = bacc.Bacc(None, target_bir_lowering=False)
    x_in = nc.dram_tensor("xt_shard", [P, ntiles * TILEB], F8, kind="ExternalInput")
    ix_in = nc.dram_tensor("idxlf", [P, ntiles], U8, kind="ExternalInput")
    out_d = nc.dram_tensor("out_shard", [SPC], F32, kind="ExternalOutput")

    w1d = nc.inline_tensor(w1dr, "w1dr")
    wf0d = nc.inline_tensor(wf0, "wf0")
    wf1d = nc.inline_tensor(wf1, "wf1")
    b2fd = nc.inline_tensor(b2f, "b2f") if has_b2 else None
    rw2d = nc.inline_tensor(rw2k, "rw2k")
    idn16d = nc.inline_tensor(idn16, "idn16")
    js2d = nc.inline_tensor(js2, "js2")
    b1d = nc.inline_tensor(b1row, "b1row") if has_b1 else None
    ones1d = nc.inline_tensor(ones1, "ones1") if has_b1 else None
    onescd = nc.inline_tensor(onesc, "onesc") if has_b2 else None
    rb1d = nc.inline_tensor(rb1k, "rb1k") if has_rb1 else None
    rb2d = nc.inline_tensor(rb2k, "rb2k") if has_rb2 else None

    with tile.TileContext(nc) as tc:
        with (
            tc.tile_pool(name="consts", bufs=1) as cpool,
            tc.tile_pool(name="xb", bufs=8) as xpool,
            tc.tile_pool(name="ixb", bufs=1) as ixpool,
            tc.tile_pool(name="h1b", bufs=6) as h1pool,
            tc.tile_pool(name="selb", bufs=6) as selpool,
            tc.tile_pool(name="rho", bufs=2) as rhopool,
            tc.tile_pool(name="ph1", bufs=3, space="PSUM") as ph1,
            tc.tile_pool(name="pseg", bufs=1, space="PSUM") as pseg,
            tc.tile_pool(name="prho", bufs=1, space="PSUM") as prho,
        ):
            # ---- early-needed data. w1dr first on the sync (HWDGE) ring,
            # then the x chunks; idn16 (first needed by rho ~40us in) is
            # issued after the first chunks. Small/late consts ride the
            # gpsimd (SWDGE) ring. Warmup matmuls depend only on an on-chip
            # memset so they start as soon as the PE queue drains its
            # preamble.
            wmt = cpool.tile([P, 2], BF, tag="wmt", name="wmt")
            nc.gpsimd.memset(wmt[:], 1.0)
            w1s = cpool.tile([P, KP, 2, DHID], F8, tag="w1s", name="w1s")
            nc.sync.dma_start(out=w1s[:], in_=w1d[:])
            idn16s = cpool.tile([P, P], BF, tag="idn", name="idn16s")
            ixall = ixpool.tile([P, ntiles], U8, tag="ixall", name="ixall")
            nc.gpsimd.dma_start(out=ixall[:], in_=ix_in[:])
            js = cpool.tile([P, 4, SBLK], U8, tag="js", name="js")
            nc.gpsimd.dma_start(out=js[:], in_=js2d[:])
            wf0s = cpool.tile_from(wf0d[:], forced_dma_engine=mybir.EngineType.Pool)
            wf1s = cpool.tile_from(wf1d[:], forced_dma_engine=mybir.EngineType.Pool)
            b2fs = cpool.tile_from(b2fd[:], forced_dma_engine=mybir.EngineType.Pool) if has_b2 else None
            rw2s = cpool.tile_from(rw2d[:], forced_dma_engine=mybir.EngineType.Pool)
            b1s = cpool.tile_from(b1d[:], forced_dma_engine=mybir.EngineType.Pool) if has_b1 else None
            ones1s = cpool.tile_from(ones1d[:], forced_dma_engine=mybir.EngineType.Pool) if has_b1 else None
            onescs = cpool.tile_from(onescd[:], forced_dma_engine=mybir.EngineType.Pool) if has_b2 else None
            rb1s = cpool.tile_from(rb1d[:], forced_dma_engine=mybir.EngineType.Pool) if has_rb1 else None
            rb2s = cpool.tile_from(rb2d[:], forced_dma_engine=mybir.EngineType.Pool) if has_rb2 else None

            # warm up the PE HAM clock gate while the first x chunks stream
            # in: wide dummy matmuls (N=512) so ~3.4us of PE busy time costs
            # few instructions and starts as soon as idn16 lands.
            # single psum bank shared by warmup + all rho scratch:
            # [:, 0:2, :] = xsum transpose, [0:6, 2, :] = rho hidden,
            # [0:1, 3, :] = rho output
            rhot = prho.tile([P, 4, P], F32, tag="rho", name="rhot")
            for _ in range(10):
                nc.tensor.matmul(
                    out=rhot[0:1, :, :],
                    lhsT=wmt[:, 0:1],
                    rhs=wmt[:, 0:1].to_broadcast([P, 1, 512]),
                    start=True,
                    stop=True,
                )
            # one psum bank, two block-alternating segment accumulators
            psegT = pseg.tile([P, 2, HW], F32, tag="seg", name="psegT")

            pending = []  # deferred seg matmuls: (selb, h1b, pr)
            rho_q = []  # deferred rho stages: callables

            def emit_seg(selq_t, h1b_t, g, pr):
                blk = pr // (tblk // 2)
                with tc.high_priority(offset=-48):
                    nc.tensor.matmul(
                    out=psegT[:, blk % 2, :],
                        lhsT=selq_t[:, 2 * g : 2 * g + 2, :],
                        rhs=h1b_t[:, 2 * g : 2 * g + 2, :],
                        start=(pr % (tblk // 2) == 0),
                        stop=(pr % (tblk // 2) == (tblk // 2) - 1),
                        perf_mode=DR,
                    )
                if pr % (tblk // 2) == (tblk // 2) - 1:
                    stage_rho(blk, psegT[:, blk % 2, :])

            def stage_rho(blk, pseg_t):
                # stage A: psum -> bf16, transpose via identity matmuls
                xsb = rhopool.tile([P, HW], BF, tag="xsb")
                xsT = rhopool.tile([P, 2, P], BF, tag="xsT")

                def stage_a0():
                    nc.vector.tensor_copy(out=xsb[:], in_=pseg_t[:])

                def stage_a1():
                    with tc.high_priority(offset=-48):
                        nc.tensor.matmul(
                            out=rhot[:, 0, :], lhsT=xsb[:, 0:P], rhs=idn16s[:],
                            start=True, stop=True,
                        )
                        nc.tensor.matmul(
                            out=rhot[0 : HW - P, 1, :], lhsT=xsb[:, P:HW],
                            rhs=idn16s[:], start=True, stop=True,
                        )

                def stage_b():
                    nc.vector.tensor_copy(out=xsT[:, 0, :], in_=rhot[:, 0, :])
                    nc.vector.tensor_copy(
                        out=xsT[0 : HW - P, 1, :], in_=rhot[0 : HW - P, 1, :]
                    )

                def stage_c():
                    # r [6, 128 segs] = relu((W2@rho_w1).T @ xsum1T (+ b2f*cnt) + rb1)
                    prT = rhot[0:6, 2, :]
                    with tc.high_priority(offset=-48):
                        nc.tensor.matmul(
                            out=prT, lhsT=wf0s[:], rhs=xsT[:, 0, :],
                            start=True, stop=False,
                        )
                        nc.tensor.matmul(
                            out=prT, lhsT=wf1s[:], rhs=xsT[0:64, 1, :],
                            start=False, stop=not has_b2,
                        )
                        if has_b2:
                            nc.tensor.matmul(
                                out=prT, lhsT=b2fs[:], rhs=xsT[64:65, 1, :],
                                start=False, stop=True,
                            )
                    rtb = rhopool.tile([6, P], BF, tag="rtb")
                    if has_rb1:
                        nc.scalar.activation(
                            out=rtb[:], in_=prT, func=Relu, bias=rb1s[:]
                        )
                    else:
                        nc.scalar.activation(out=rtb[:], in_=prT, func=Relu)
                    pot = rhot[0:1, 3, :]
                    nc.tensor.matmul(
                        out=pot, lhsT=rw2s[:], rhs=rtb[:], start=True, stop=True
                    )
                    ob = rhopool.tile([1, P], F32, tag="ob")
                    if has_rb2:
                        nc.scalar.activation(out=ob[:], in_=pot, func=Copy, bias=rb2s[:])
                    else:
                        nc.scalar.activation(out=ob[:], in_=pot, func=Copy)
                    nc.sync.dma_start(
                        out=out_d[blk * SBLK : (blk + 1) * SBLK], in_=ob[:]
                    )

                rho_q.append(stage_a0)
                rho_q.append(stage_a1)
                rho_q.append(stage_b)
                rho_q.append(stage_c)

            toff = 0  # tile offset of current chunk
            boff = 0  # byte offset into x_in per partition
            for ch, ctiles in enumerate(chunk_tiles):
                # ramp: first chunk in pieces so the PE starts early while
                # keeping per-partition DMA lines >= 768B
                xb = xpool.tile([P, TPC, KP, 2, P], F8, tag="xb", name=f"xb_{ch}")
                if ch == 0:
                    splits = [4, 4] + ([8] if ctiles == TPC else [])
                elif ch < 3:
                    splits = [8, 8] if ctiles == TPC else [8]
                else:
                    splits = [ctiles]
                n0 = 0
                for tq in splits:
                    nc.sync.dma_start(
                        out=xb[:, n0 : n0 + tq, :, :, :],
                        in_=x_in[
                            :, boff + n0 * TILEB : boff + (n0 + tq) * TILEB
                        ].rearrange("p (n c3 i r) -> p n c3 i r", n=tq, c3=KP, i=2),
                    )
                    n0 += tq
                if ch == 2:
                    nc.sync.dma_start(out=idn16s[:], in_=idn16d[:])

                for q in range(ctiles // 4):
                    t0 = toff + q * 4  # global tile index of quad start
                    pr0 = t0 // 2  # global pair index
                    ph1t = ph1.tile([P, 4, 256], F32, tag="h1", name=f"ph1_{t0}")
                    for j in range(4):
                        n = q * 4 + j
                        for c3 in range(KP):
                            nc.tensor.matmul(
                                out=ph1t[:, j, 0:DHID],
                                lhsT=xb[:, n, c3, :, :],
                                rhs=w1s[:, c3, :, :],
                                start=(c3 == 0),
                                stop=(c3 == KP - 1 and not has_b1),
                                perf_mode=DR,
                            )
                        if has_b1:
                            nc.tensor.matmul(
                                out=ph1t[:, j, 0:DHID], lhsT=ones1s[:], rhs=b1s[:],
                                start=False, stop=True,
                            )
                        # interleave deferred seg MMs / rho stages between
                        # mm1 groups so their input latencies are hidden
                        if j % 2 == 0:
                            if len(pending) > DEFER_PAIRS:
                                emit_seg(*pending.pop(0))
                        elif rho_q:
                            rho_q.pop(0)()
                    h1b = h1pool.tile([P, 4, HW], F8, tag="h1b", name=f"h1b_{t0}")
                    # relu + 1/SCALE + fp8 cast, one ACT instr per quad
                    nc.scalar.activation(
                        out=h1b[:, :, 0:DHID], in_=ph1t[:, :, 0:DHID], func=Relu,
                        scale=1.0 / SCALE,
                    )
                    if has_b2:
                        nc.vector.tensor_copy(
                            out=h1b[:, :, DHID:HW],
                            in_=onescs[:].to_broadcast([P, 4, 1]),
                        )
                    selq = selpool.tile([P, 4, SBLK], F8, tag="selb", name=f"sel_{t0}")
                    nc.vector.tensor_tensor(
                        out=selq[:],
                        in0=ixall[:, t0 : t0 + 4].to_broadcast([P, 4, SBLK]),
                        in1=js[:],
                        op=Alu.is_equal,
                    )
                    pending.append((selq, h1b, 0, pr0))
                    pending.append((selq, h1b, 1, pr0 + 1))
                toff += ctiles
                boff += ctiles * TILEB
            while pending:
                emit_seg(*pending.pop(0))
            while rho_q:
                rho_q.pop(0)()

    nc.compile()
    return nc


_CACHE = {}


def _get_nc(tblk, weights):
    key = tblk
    if key not in _CACHE:
        if USE_LDW_OPT:
            import concourse.bass_utils as bu

            orig = bu.run_command

            def run_command_ldwopt(argv, **kw):
                argv = [
                    "--enable-ldw-opt=true" if a == "--enable-ldw-opt=false" else a
                    for a in argv
                ]
                return orig(argv, **kw)

            bu.run_command = run_command_ldwopt
            try:
                _CACHE[key] = _build(tblk, *weights)
            finally:
                bu.run_command = orig
        else:
            _CACHE[key] = _build(tblk, *weights)
    return _CACHE[key]


def _run(inputs, trace=False):
    from concourse.bass_utils import run_bass_kernel_spmd

    inp = {k: np.asarray(v) for k, v in inputs.items()}
    x = inp["x"].astype(f32, copy=False)
    idx = inp["idx"].astype(np.int32, copy=False)
    weights = tuple(
        inp[k].astype(f32, copy=False)
        for k in ("phi_w1", "phi_b1", "phi_w2", "phi_b2", "rho_w1", "rho_b1", "rho_w2", "rho_b2")
    )
    xs_t, ixs, tblk, counts, loc = _prep(x, idx)
    nc = _get_nc(tblk, weights)
    in_maps = [{"xt_shard": xs_t[c], "idxlf": ixs[c]} for c in range(NCORES)]
    res = run_bass_kernel_spmd(nc, in_maps, core_ids=list(range(NCORES)), trace=trace)
    res_flat = np.concatenate([res.results[c]["out_shard"] for c in range(NCORES)])
    out = res_flat[loc].reshape(B, 1).astype(f32)
    # safety net: empty segments (never happens for the target distribution)
    if np.any(counts == 0):
        (phi_w1, phi_b1, phi_w2, phi_b2, rho_w1, rho_b1, rho_w2, rho_b2) = weights
        z = np.zeros((1, DHID), f32)
        r = np.maximum(z @ rho_w1 + rho_b1, 0.0)
        o0 = (r @ rho_w2 + rho_b2).astype(f32)
        out[counts == 0] = o0
    return out, res


def kernel(**inputs) -> np.ndarray:
    return _run(inputs, trace=False)[0]


if __name__ == "__main__":
    # quick self-test against numpy
    rng = np.random.default_rng(0)
    x = rng.standard_normal((N, DIN)).astype(f32)
    idx = np.sort(rng.integers(0, B, N).astype(np.int32))
    w1 = (rng.standard_normal((DIN, DHID)) / np.sqrt(DIN)).astype(f32)
    w2 = (rng.standard_normal((DHID, DHID)) / np.sqrt(DHID)).astype(f32)
    r1 = (rng.standard_normal((DHID, 6)) / np.sqrt(DHID)).astype(f32)
    r2 = (rng.standard_normal((6, 1)) / np.sqrt(6)).astype(f32)
    inputs = dict(
        x=x, idx=idx,
        phi_w1=w1, phi_b1=np.zeros(DHID, f32), phi_w2=w2, phi_b2=np.zeros(DHID, f32),
        rho_w1=r1, rho_b1=np.zeros(6, f32), rho_w2=r2, rho_b2=np.zeros(1, f32),
    )
    out = kernel(**inputs)
    h = np.maximum(x @ w1, 0.0) @ w2
    xsum = np.zeros((B, DHID), f32)
    np.add.at(xsum, idx, h)
    exp = np.maximum(xsum @ r1, 0.0) @ r2
    rel = np.linalg.norm(out - exp) / np.linalg.norm(exp)
    print("self-test rel err:", rel)
